# revision 3
# baseline (speedup 1.0000x reference)
"""Trainium2 Bass kernel for nn_DepthwiseConvOverTimeLayer.

Pipeline (per core, C-sharded 8 ways), v2 "sd-drain":
  stage A: depthwise 3x3 conv as per-channel banded matmul on PE,
           fp8e4m3 DoubleRow (K=49 split 25+24, + bias pair row folding
           dw_b), emitting per (b, t-pair):
             s = y(odd t)          [psum, group left OPEN]
             d = y(even) - y(odd)  [psum, closed]  (host preps x-diffs)
  drain:   ACT: r = relu(d) -> SBUF fp16 (one op per 2-ch tile)
           PE:  identity-matmul accumulates r onto s  =>  m1 = max pair
           DVE: segmented tensor_reduce max over the 10 pairs -> ymax
           (a-class tiles instead do ACT full copy + DVE fp16 tree)
  stage B: PE transpose per b -> ymT [ch, b, m, 9x9-padded halo].
  stage C: conv2 as 36 block-diag matmuls (fp16, 9-tap PSUM accum),
           conv_b added via ACT Identity with per-partition bias AP.

Channel coupling: conv2 group c2 consumes depthwise channel
c = 256*m2 + c2//4 at multiplier m = c2 % 4; core j owns c2 in
[128j, 128j+128) -> dw channels {256*(l//32) + 32j + (l%32)}.
"""

import numpy as np
import ml_dtypes

B, T, H, W, C, M = 16, 20, 7, 7, 1024, 4
KD = KP = 3
NCORES = 8
CL = 128          # dw channels per core
HWQ = 49
BT = B * T        # 320
NT = CL // 2      # 2-channel tiles per core
F16 = np.float16
F8 = ml_dtypes.float8_e4m3fn

# drain class per 2-ch tile: "S" = sd (PE-assisted)
STRIPE = ["S"] * NT

TRACE = False
LAST_RESULTS = None
_NC_CACHE = None


# ----------------------------------------------------------------- host prep
def _core_channels(j):
    l = np.arange(CL)
    return 256 * (l // 32) + 32 * j + (l % 32)


def _col_of(m, ho, wo):
    """Stage-A output row index: A-half (ho<4) then B-half, m-major."""
    if ho < 4:
        return m * 28 + ho * 7 + wo
    return 112 + m * 21 + (ho - 4) * 7 + wo


def build_core_inputs(x, dw_w, dw_b, conv_w, conv_b, j):
    cj = _core_channels(j)

    # --- xb [26, 2, CL, 320] fp8: cols 0:160 = s (odd t), 160:320 = d ---
    xs = np.asarray(x, np.float32)[:, :, :, :, cj]           # [B,T,H,W,CL]
    xf = xs.transpose(0, 1, 4, 2, 3).reshape(B, T, CL, HWQ)  # [B,T,CL,49]
    xodd = xf[:, 1::2]                                       # [B,10,CL,49]
    xdif = xf[:, 0::2] - xodd
    # [49(+1 pad), CL, 320]: col = b*10+i | 160 + b*10+i
    xb = np.zeros((52, CL, 320), np.float32)
    xb[0:HWQ, :, 0:160] = xodd.transpose(3, 2, 0, 1).reshape(HWQ, CL, 160)
    xb[0:HWQ, :, 160:320] = xdif.transpose(3, 2, 0, 1).reshape(HWQ, CL, 160)
    xb[50, :, 0:160] = 1.0                                   # bias rows: s only
    xb[51, :, 0:160] = 1.0
    xb = xb.reshape(26, 2, CL, 320)

    # --- aw [26, 2, CL, 112|112] fp8 banded depthwise lhsT + bias row ---
    wsel = np.asarray(dw_w, np.float32)[:, :, cj, :]         # [3,3,CL,4]
    aw = np.zeros((52, CL, 224), np.float32)
    for ho in range(H):
        for wo in range(W):
            for kh in range(KD):
                for kw in range(KD):
                    hi, wi = ho + kh - 1, wo + kw - 1
                    if 0 <= hi < H and 0 <= wi < W:
                        for m in range(M):
                            aw[hi * 7 + wi, :, _col_of(m, ho, wo)] = \
                                wsel[kh, kw, :, m]
    # bias pair rows (50, 51): dw_b[c*M + m] split hi+lo for fp8 precision
    dwb = np.asarray(dw_b, np.float32).reshape(C, M)[cj]     # [CL, 4]
    for m in range(M):
        b_hi = dwb[:, m].astype(F8).astype(np.float32)
        for ho in range(H):
            for wo in range(W):
                col = _col_of(m, ho, wo)
                aw[50, :, col] = b_hi
                aw[51, :, col] = dwb[:, m] - b_hi
    aw = aw.reshape(26, 2, CL, 224)
    awA = np.ascontiguousarray(aw[:, :, :, 0:112])
    awB = np.ascontiguousarray(aw[:, :, :, 112:224])   # data 0:84, pad 84:112

    # --- a2 [CL, 36, CL] f16: conv2 block-diag weights (natural order) ---
    # a2[p_in=32*m2+s, t9*4+r, p_out=4*s+mo] = conv_w[c2=128j+4s+r, kh, kw, m2, mo]
    a2 = np.zeros((CL, 36, CL), np.float32)
    cw = np.asarray(conv_w, np.float32)
    s = np.arange(32)
    for t9 in range(9):
        kh, kw = divmod(t9, 3)
        for r in range(4):
            blk = cw[128 * j + 4 * s + r, kh, kw, :, :]       # [32, m2, mo]
            for m2 in range(4):
                a2[32 * m2 + s[:, None], t9 * 4 + r,
                   4 * s[:, None] + np.arange(4)[None, :]] = blk[:, m2, :]

    # --- zc [CL, 4] f32: conv_b per stage-C psum partition, per r ---
    cb = np.asarray(conv_b, np.float32)
    c2s = 128 * j + np.arange(CL)
    zc = np.empty((CL, 4), np.float32)
    for mo in range(4):
        for r in range(4):
            zc[4 * s + mo, r] = cb[c2s[4 * s + r], mo]

    ident = np.eye(112, dtype=np.float32)

    return {"xb": xb.astype(F8), "awA": awA.astype(F8), "awB": awB.astype(F8),
            "a2": a2.astype(F16), "zc": zc,
            "ident": ident.astype(F16)}


def assemble_output(core_outs):
    """core_outs[j]['zout'] [CL=(4s+mo), 4=r, 16=b, 49=hw] -> (B,M,H,W,C)."""
    out = np.empty((B, M, H, W, C), np.float32)
    for j in range(NCORES):
        z = np.asarray(core_outs[j]["zout"]).astype(np.float32)
        z = z.reshape(32, 4, 4, B, HWQ)                 # s, mo, r, b, hw
        zz = z.transpose(3, 1, 4, 0, 2).reshape(B, M, H, W, CL)
        out[:, :, :, :, 128 * j:128 * j + 128] = zz
    return out


# ----------------------------------------------------------------- bass build
def build_bass():
    import concourse.mybir as mybir
    from concourse import bacc
    from concourse.tile import TileContext

    dt = mybir.dt
    op = mybir.AluOpType
    DR = mybir.MatmulPerfMode.DoubleRow
    AF = mybir.ActivationFunctionType
    AX = mybir.AxisListType
    nc = bacc.Bacc()

    xb_d = nc.dram_tensor("xb", [26, 2, CL, 320], dt.float8e4,
                          kind="ExternalInput")
    awA_d = nc.dram_tensor("awA", [26, 2, CL, 112], dt.float8e4,
                           kind="ExternalInput")
    awB_d = nc.dram_tensor("awB", [26, 2, CL, 112], dt.float8e4,
                           kind="ExternalInput")
    a2_d = nc.dram_tensor("a2", [CL, 36, CL], dt.float16, kind="ExternalInput")
    zc_d = nc.dram_tensor("zc", [CL, 4], dt.float32, kind="ExternalInput")
    ident_d = nc.dram_tensor("ident", [112, 112], dt.float16,
                             kind="ExternalInput")
    zout_d = nc.dram_tensor("zout", [CL, 4, B, HWQ], dt.float16,
                            kind="ExternalOutput")

    def tt_max(eng, out, in0, in1):
        eng.add_instruction(mybir.InstTensorTensor(
            name=nc.get_next_instruction_name(),
            ins=[eng.lower_ap(in0), eng.lower_ap(in1)],
            outs=[eng.lower_ap(out)], op=op.max))

    with TileContext(nc) as tc:
        with tc.tile_pool(name="const", bufs=1) as cpool:
            xb_t = cpool.tile([26, 2, CL, 320], dt.float8e4)
            awA_t = cpool.tile([26, 2, CL, 112], dt.float8e4)
            awB_t = cpool.tile([26, 2, CL, 112], dt.float8e4)
            a2_t = cpool.tile([CL, 36, CL], dt.float16)
            zc_t = cpool.tile([CL, 4], dt.float32)
            ident_t = cpool.tile([112, 112], dt.float16)
            ymax = cpool.tile([112, CL, 2, B], dt.float16)  # [row, ch, A|B, b]
            ymT = cpool.tile([CL, B, 4, 81], dt.float16)    # [ch, b, m, 9x9]
            zsb = cpool.tile([CL, 2, 8, HWQ], dt.float16)
            rr = cpool.tile([112, 2, 4, B, 10], dt.float16)  # relu ring

            # input DMAs, staggered by channel blocks for early start
            nc.sync.dma_start(out=awA_t[:, :, 0:32], in_=awA_d[:, :, 0:32])
            nc.sync.dma_start(out=awB_t[:, :, 0:32], in_=awB_d[:, :, 0:32])
            nc.sync.dma_start(out=xb_t[:, :, 0:32], in_=xb_d[:, :, 0:32])
            nc.sync.dma_start(out=ident_t[:], in_=ident_d[:])
            for c0 in range(32, CL, 32):
                nc.sync.dma_start(out=awA_t[:, :, c0:c0 + 32],
                                  in_=awA_d[:, :, c0:c0 + 32])
                nc.sync.dma_start(out=awB_t[:, :, c0:c0 + 32],
                                  in_=awB_d[:, :, c0:c0 + 32])
                nc.sync.dma_start(out=xb_t[:, :, c0:c0 + 32],
                                  in_=xb_d[:, :, c0:c0 + 32])
            nc.sync.dma_start(out=a2_t[:], in_=a2_d[:])
            nc.sync.dma_start(out=zc_t[:], in_=zc_d[:])

            # halo borders of ymT (interior is fully written in stage B)
            ymg = ymT.rearrange("p b m (hh ww) -> p b m hh ww", hh=9, ww=9)
            nc.gpsimd.memset(ymg[:, :, :, 0, :], 0.0)
            nc.gpsimd.memset(ymg[:, :, :, 8, :], 0.0)
            nc.gpsimd.memset(ymg[:, :, :, 1:8, 0], 0.0)
            nc.gpsimd.memset(ymg[:, :, :, 1:8, 8], 0.0)

            # ---------------- stage A + drain: 64 2-ch tiles
            with tc.tile_pool(name="psA", bufs=2, space="PSUM") as psA:
                def emit_tile(i, pz):
                    """8 matmuls for tile i: d-parts then s-parts."""
                    for ch in range(2):
                        g = 2 * i + ch
                        for h in range(2):
                            bank = 2 * ch + h
                            awp = (awA_t if h == 0 else awB_t)[:, :, g]
                            nc.tensor.matmul(
                                pz[0:112, bank, 160:320], awp,
                                xb_t[:, :, g, 160:320],
                                start=True, stop=True, perf_mode=DR)
                    for ch in range(2):
                        g = 2 * i + ch
                        for h in range(2):
                            bank = 2 * ch + h
                            awp = (awA_t if h == 0 else awB_t)[:, :, g]
                            nc.tensor.matmul(
                                pz[0:112, bank, 0:160], awp,
                                xb_t[:, :, g, 0:160],
                                start=True, stop=False, perf_mode=DR)

                def drain_relu(i, pz):
                    nc.scalar.activation(
                        rr[:, i % 2],
                        pz[:, 0:4, 160:320].rearrange(
                            "p k (b t) -> p k b t", t=10),
                        AF.Relu)

                def drain_finish(i, pz):
                    for bank in range(4):
                        nc.tensor.matmul(
                            pz[0:112, bank, 0:160], ident_t[:],
                            rr[:, i % 2, bank].rearrange("p b t -> p (b t)"),
                            start=False, stop=True)
                    nc.vector.tensor_reduce(
                        ymax[:, 2 * i:2 * i + 2],
                        pz[:, 0:4, 0:160].rearrange(
                            "p (c h) (b t) -> p c h b t", c=2, t=10),
                        AX.X, op.max)

                prev = [None]
                for i in range(NT):
                    pz = psA.tile([112, 4, 512], dt.float32)
                    emit_tile(i, pz)
                    if prev[0] is not None:
                        drain_finish(*prev[0])
                    drain_relu(i, pz)
                    prev[0] = (i, pz)
                drain_finish(*prev[0])

            # ---------------- stage B + C per b-half
            with tc.tile_pool(name="psT", bufs=2, space="PSUM") as psT, \
                 tc.tile_pool(name="psC", bufs=2, space="PSUM") as psC:
                for bh in range(2):
                    for b in range(8 * bh, 8 * bh + 8):
                        tt = psT.tile([CL, 196], dt.float16, tag="tt")
                        nc.tensor.transpose(
                            tt[:, 0:112], ymax[:, :, 0, b], ident_t[:])
                        nc.tensor.transpose(
                            tt[:, 112:196], ymax[0:84, :, 1, b],
                            ident_t[0:84, 0:84])
                        dsta = ymg[:, b, :, 1:5, 1:8]
                        srca = tt[:, 0:112].rearrange(
                            "p (m h w) -> p m h w", m=4, w=7)
                        dstb = ymg[:, b, :, 5:8, 1:8]
                        srcb = tt[:, 112:196].rearrange(
                            "p (m h w) -> p m h w", m=4, w=7)
                        if b % 2 == 0:
                            nc.vector.tensor_scalar_add(dsta, srca, 0.0)
                            nc.vector.tensor_scalar_add(dstb, srcb, 0.0)
                        else:
                            nc.scalar.copy(dsta, srca)
                            nc.scalar.copy(dstb, srcb)

                    for r in range(4):
                        pzc = psC.tile([CL, 8, HWQ], dt.float32)
                        for t9 in range(9):
                            kh, kw = divmod(t9, 3)
                            rhs = ymg[:, 8 * bh:8 * bh + 8, r,
                                      kh:kh + 7, kw:kw + 7]
                            nc.tensor.matmul(pzc[:], a2_t[:, 4 * t9 + r, :],
                                             rhs, start=(t9 == 0),
                                             stop=(t9 == 8))
                        zslice = zsb[:, r % 2]
                        nc.scalar.activation(zslice, pzc[:], AF.Identity,
                                             bias=zc_t[:, r:r + 1], scale=1.0)
                        nc.sync.dma_start(
                            out=zout_d[:, r, 8 * bh:8 * bh + 8, :], in_=zslice)

    nc.finalize()
    return nc


def _get_nc():
    global _NC_CACHE
    if _NC_CACHE is None:
        _NC_CACHE = build_bass()
    return _NC_CACHE


# ----------------------------------------------------------------- entry point
def kernel(x, dw_w, dw_b, conv_w, conv_b):
    global LAST_RESULTS
    from concourse.bass_utils import run_bass_kernel_spmd

    in_maps = [build_core_inputs(x, dw_w, dw_b, conv_w, conv_b, j)
               for j in range(NCORES)]
    nc = _get_nc()
    res = run_bass_kernel_spmd(nc, in_maps, core_ids=list(range(NCORES)),
                               trace=TRACE)
    LAST_RESULTS = res
    return assemble_output(res.results)
